# revision 1
# baseline (speedup 1.0000x reference)
"""AQLM 1x16 quantized linear on 8 trn2 NeuronCores.

y = x @ dequant(codes, codebook, scales).T + bias
  x:         [64, 4096]  f32
  codes:     [11008, 512, 1] int32 in [0, 65536)
  codebooks: [1, 65536, 1, 8] f32
  scales:    [11008, 1, 1, 1] f32
  bias:      [11008] f32
  out:       [64, 11008] f32

Sharding: out_features across 8 cores (1376 rows each, padded to 1408).

Per-core pipeline (all in one Tile program):
  - codebook stored in HBM as paired rows: row r = [CB[2r](8), CB[2r+1](8), 0*48]
    (256B rows so dma_gather's int16 index c>>1 covers all 65536 entries)
  - dma_gather (SWDGE, 4 queues) pulls one 256B row per code into SBUF
  - DVE select picks the even/odd entry by parity mask (host-precomputed)
  - PE transpose turns W[o,k] blocks into W.T[k,o]
  - PE matmul accumulates y[64, o-block] over all 32 k-chunks in PSUM
  - y = y*scales + bias on DVE, DMA out
"""

import sys

sys.path.insert(0, "/opt/trn_rl_repo")

import numpy as np

N_CORES = 8
TOKENS = 64
IN_F = 4096
OUT_F = 11008
IN_GROUP = 8
N_IG = IN_F // IN_GROUP          # 512 groups
CB_SIZE = 65536

O_SHARD = OUT_F // N_CORES       # 1376
O_PAD = 1408                     # 11 * 128
N_OB = O_PAD // 128              # 11 out-row blocks per core
import os
JC = int(os.environ.get('AQLM_JC', '32'))
N_JC = N_IG // JC                # 16 gather calls per out-row block
CALL_IDX = 128 * JC              # 4096 indices per gather call
ELEM = 64                        # gathered row: 64 f32 = 256B
TAB_ROWS = CB_SIZE // 2          # 32768 paired rows

_CACHED = {}


def _build_program():
    import os
    import concourse.bacc as bacc
    import concourse.mybir as mybir
    import concourse.tile as tile
    from concourse.bass import ts
    from concourse.masks import make_identity

    nc = bacc.Bacc("TRN2", target_bir_lowering=False, debug=False,
                   num_devices=1, num_swdge_queues=4,
                   dynamic_dma_scratch_size=int(os.environ.get("AQLM_SCRATCH", "16384")))
    dt = mybir.dt

    NTAB = int(os.environ.get("AQLM_NTAB", "1"))
    tab_d = nc.dram_tensor("tab", [NTAB, TAB_ROWS, ELEM], dt.float32,
                           kind="ExternalInput")
    # wrapped int16 indices, replicated across all 128 partitions:
    # [block, call, 128, CALL_IDX//16]
    IDX_ROWS = 128
    idx_d = nc.dram_tensor("idx", [N_OB, N_JC, IDX_ROWS, CALL_IDX // 16], dt.int16,
                           kind="ExternalInput")
    # parity mask expanded over the 8 components: [block, 128, N_IG*8]
    MSK_EXP = False
    msk_d = nc.dram_tensor(
        "msk", [N_OB, 128, N_IG * IN_GROUP if MSK_EXP else N_IG], dt.float32,
        kind="ExternalInput")
    # x.T packed per k-chunk: [128, 32*64], col (kc, t)
    xt_d = nc.dram_tensor("xt", [128, (IN_F // 128) * TOKENS], dt.float32,
                          kind="ExternalInput")
    scl_d = nc.dram_tensor("scl", [TOKENS, O_PAD], dt.float32, kind="ExternalInput")
    bia_d = nc.dram_tensor("bia", [TOKENS, O_PAD], dt.float32, kind="ExternalInput")
    y_d = nc.dram_tensor("y", [TOKENS, O_PAD], dt.float32, kind="ExternalOutput")

    with tile.TileContext(nc) as tc:
        with (
            tc.tile_pool(name="const", bufs=1) as cpool,
            tc.tile_pool(name="idx", bufs=int(os.environ.get("AQLM_IBUFS", "3"))) as ipool,
            tc.tile_pool(name="msk", bufs=2) as mpool,
            tc.tile_pool(name="gath", bufs=int(os.environ.get("AQLM_GBUFS", "8"))) as gpool,
            tc.tile_pool(name="w", bufs=4) as wpool,
            tc.tile_pool(name="wt", bufs=4) as wtpool,
            tc.tile_pool(name="y", bufs=2) as ypool,
            tc.tile_pool(name="pt", bufs=4, space="PSUM") as ptpool,
            tc.tile_pool(name="py", bufs=2, space="PSUM") as pypool,
        ):
            xt = cpool.tile([128, (IN_F // 128) * TOKENS], dt.float32)
            nc.sync.dma_start(xt[:], xt_d.ap())
            scl = cpool.tile([TOKENS, O_PAD], dt.float32)
            nc.sync.dma_start(scl[:], scl_d.ap())
            bia = cpool.tile([TOKENS, O_PAD], dt.float32)
            nc.sync.dma_start(bia[:], bia_d.ap())
            ident = cpool.tile([128, 128], dt.float32)
            make_identity(nc, ident[:])

            qn = 0
            for b in range(N_OB):
                # per-block index + mask staging
                idxb = ipool.tile([128, N_JC * (CALL_IDX // 16)], dt.int16, tag="idx")
                if IDX_ROWS == 128:
                    nc.sync.dma_start(
                        idxb[:].rearrange("p (c w) -> p c w", w=CALL_IDX // 16),
                        idx_d.ap()[b].rearrange("c p w -> p c w"),
                    )
                else:
                    nc.sync.dma_start(
                        idxb[16:32].rearrange("p (c w) -> p c w", w=CALL_IDX // 16),
                        idx_d.ap()[b].rearrange("c p w -> p c w"),
                    )
                mskb = mpool.tile(
                    [128, N_IG * IN_GROUP if MSK_EXP else N_IG], dt.float32, tag="msk")
                nc.sync.dma_start(mskb[:], msk_d.ap()[b])

                ypsum = pypool.tile([TOKENS, 128], dt.float32, tag="ypsum")

                for jc in range(N_JC):
                    gath = gpool.tile([128, JC, ELEM], dt.float32, tag="gath")
                    nc.gpsimd.dma_gather(
                        gath[:],
                        tab_d.ap()[qn % NTAB],
                        idxb[:, ts(jc, CALL_IDX // 16)],
                        num_idxs=CALL_IDX,
                        num_idxs_reg=CALL_IDX,
                        elem_size=ELEM,
                        single_packet=False,
                        queue_num=qn % 4,
                    )
                    qn += 1

                    ge = gath[:, :, 0:IN_GROUP]
                    go = gath[:, :, IN_GROUP:2 * IN_GROUP]
                    if MSK_EXP:
                        m8 = mskb[:, ts(jc, JC * IN_GROUP)].rearrange(
                            "p (j c) -> p j c", c=IN_GROUP)
                    else:
                        m8 = mskb[:, ts(jc, JC)].rearrange(
                            "p (j one) -> p j one", one=1).broadcast_to(
                            [128, JC, IN_GROUP])

                    wd = wpool.tile([128, JC, IN_GROUP], dt.float32, tag="wd")
                    nc.vector.tensor_tensor(
                        out=wd[:], in0=go, in1=ge, op=mybir.AluOpType.subtract)
                    nc.vector.tensor_tensor(
                        out=wd[:], in0=wd[:], in1=m8, op=mybir.AluOpType.mult)
                    w8 = wpool.tile([128, JC * IN_GROUP], dt.float32, tag="w8")
                    nc.vector.tensor_tensor(
                        out=w8[:].rearrange("p (j c) -> p j c", c=IN_GROUP),
                        in0=wd[:], in1=ge, op=mybir.AluOpType.add)

                    # two 128-col sub-blocks -> transpose -> matmul accumulate
                    for h in range(JC * IN_GROUP // 128):
                        pt = ptpool.tile([128, 128], dt.float32, tag="pt")
                        nc.tensor.transpose(pt[:], w8[:, ts(h, 128)], ident[:])
                        wt = wtpool.tile([128, 128], dt.float32, tag="wt")
                        nc.vector.tensor_copy(wt[:], pt[:])
                        kc = jc * (JC * IN_GROUP // 128) + h
                        nc.tensor.matmul(
                            ypsum[:],
                            xt[:, ts(kc, TOKENS)],
                            wt[:],
                            start=(kc == 0),
                            stop=(kc == IN_F // 128 - 1),
                        )

                # y = ypsum * scales + bias
                ysb = ypool.tile([TOKENS, 128], dt.float32, tag="ysb")
                nc.vector.tensor_tensor(
                    out=ysb[:], in0=ypsum[:],
                    in1=scl[:, ts(b, 128)],
                    op=mybir.AluOpType.mult)
                nc.vector.tensor_tensor(
                    out=ysb[:], in0=ysb[:],
                    in1=bia[:, ts(b, 128)],
                    op=mybir.AluOpType.add)
                nc.sync.dma_start(y_d.ap()[:, ts(b, 128)], ysb[:])

    nc.compile()
    return nc


def _host_prep(x, codes, codebooks, scales, bias):
    """Build the paired-row table plus per-core index/mask/x tensors."""
    import os as _os
    cb = codebooks[0, :, 0, :].astype(np.float32)          # [65536, 8]
    tab1 = np.zeros((TAB_ROWS, ELEM), np.float32)
    tab1[:, 0:IN_GROUP] = cb[0::2]
    tab1[:, IN_GROUP:2 * IN_GROUP] = cb[1::2]
    ntab = int(_os.environ.get("AQLM_NTAB", "1"))
    tab = np.ascontiguousarray(np.broadcast_to(tab1, (ntab, TAB_ROWS, ELEM)))

    xt = np.ascontiguousarray(x.T).reshape(IN_F // 128, 128, TOKENS)
    xt = np.ascontiguousarray(xt.transpose(1, 0, 2)).reshape(
        128, (IN_F // 128) * TOKENS).astype(np.float32)

    c_all = codes[:, :, 0].astype(np.int64)                # [11008, 512]

    in_maps = []
    for core in range(N_CORES):
        c = np.zeros((O_PAD, N_IG), np.int64)
        c[:O_SHARD] = c_all[core * O_SHARD:(core + 1) * O_SHARD]
        idx = (c >> 1).astype(np.int16)                    # [1408, 512]
        par = (c & 1).astype(np.float32)

        # wrapped gather indices: [block, call, 16, CALL_IDX//16] -> tile to 128
        # slot i of call (b, jc): i = j*128 + p  <->  (o = 128b+p, g = 32jc+j)
        iv = idx.reshape(N_OB, 128, N_JC, JC)              # [b, p, jc, j]
        iv = iv.transpose(0, 2, 3, 1).reshape(N_OB, N_JC, CALL_IDX)  # i = j*128+p
        iv = iv.reshape(N_OB, N_JC, CALL_IDX // 16, 16).transpose(0, 1, 3, 2)
        idx_t = np.ascontiguousarray(np.tile(iv, (1, 1, 8, 1)))

        # parity mask expanded over components: [b, 128, 512*8]
        # layout must match w8 columns: col (jc, j, comp) for partition p = o-row
        mv = par.reshape(N_OB, 128, N_IG)                  # [b, p, g]
        msk = np.ascontiguousarray(mv)

        s = np.zeros((1, O_PAD), np.float32)
        s[0, :O_SHARD] = scales[core * O_SHARD:(core + 1) * O_SHARD, 0, 0, 0]
        s = np.ascontiguousarray(np.tile(s, (TOKENS, 1)))
        bi = np.zeros((1, O_PAD), np.float32)
        bi[0, :O_SHARD] = bias[core * O_SHARD:(core + 1) * O_SHARD]
        bi = np.ascontiguousarray(np.tile(bi, (TOKENS, 1)))

        in_maps.append({
            "tab": tab,
            "idx": idx_t,
            "msk": msk,
            "xt": xt,
            "scl": s,
            "bia": bi,
        })
    return in_maps


def kernel(x, codes, codebooks, scales, bias):
    from concourse import bass_utils

    x = np.asarray(x)
    codes = np.asarray(codes)
    codebooks = np.asarray(codebooks)
    scales = np.asarray(scales)
    bias = np.asarray(bias)

    if "nc" not in _CACHED:
        _CACHED["nc"] = _build_program()
    nc = _CACHED["nc"]

    in_maps = _host_prep(x, codes, codebooks, scales, bias)
    res = bass_utils.run_bass_kernel_spmd(
        nc, in_maps, core_ids=list(range(N_CORES)))
    _CACHED["last_results"] = res

    out = np.empty((TOKENS, OUT_F), np.float32)
    for core in range(N_CORES):
        out[:, core * O_SHARD:(core + 1) * O_SHARD] = \
            res.results[core]["y"][:, :O_SHARD]
    return out



# revision 2
# speedup vs baseline: 1.1210x; 1.1210x over previous
"""AQLM 1x16 quantized linear on 8 trn2 NeuronCores.

y = x @ dequant(codes, codebook, scales).T + bias
  x:         [64, 4096]  f32
  codes:     [11008, 512, 1] int32 in [0, 65536)
  codebooks: [1, 65536, 1, 8] f32
  scales:    [11008, 1, 1, 1] f32
  bias:      [11008] f32
  out:       [64, 11008] f32

Sharding: out_features across 8 cores (1376 rows each, padded to 1408).

Per-core pipeline (all in one Tile program):
  - codebook stored in HBM as paired rows: row r = [CB[2r](8), CB[2r+1](8), 0*48]
    (256B rows so dma_gather's int16 index c>>1 covers all 65536 entries)
  - dma_gather (SWDGE, 4 queues) pulls one 256B row per code into SBUF
  - DVE select picks the even/odd entry by parity mask (host-precomputed)
  - PE transpose turns W[o,k] blocks into W.T[k,o]
  - PE matmul accumulates y[64, o-block] over all 32 k-chunks in PSUM
  - y = y*scales + bias on DVE, DMA out
"""

import sys

sys.path.insert(0, "/opt/trn_rl_repo")

import numpy as np

N_CORES = 8
TOKENS = 64
IN_F = 4096
OUT_F = 11008
IN_GROUP = 8
N_IG = IN_F // IN_GROUP          # 512 groups
CB_SIZE = 65536

O_SHARD = OUT_F // N_CORES       # 1376
O_PAD = 1408                     # 11 * 128
N_OB = O_PAD // 128              # 11 out-row blocks per core
import os
JC = int(os.environ.get('AQLM_JC', '32'))
N_JC = N_IG // JC                # 16 gather calls per out-row block
CALL_IDX = 128 * JC              # 4096 indices per gather call
ELEM = 64                        # gathered row: 64 f32 = 256B
TAB_ROWS = CB_SIZE // 2          # 32768 paired rows

_CACHED = {}


def _build_program():
    import os
    import concourse.bacc as bacc
    import concourse.mybir as mybir
    import concourse.tile as tile
    from concourse.bass import ts
    from concourse.masks import make_identity

    nc = bacc.Bacc("TRN2", target_bir_lowering=False, debug=False,
                   num_devices=1, num_swdge_queues=4,
                   dynamic_dma_scratch_size=int(os.environ.get("AQLM_SCRATCH", "16384")))
    dt = mybir.dt

    NTAB = int(os.environ.get("AQLM_NTAB", "1"))
    tab_d = nc.dram_tensor("tab", [NTAB, TAB_ROWS, ELEM], dt.float32,
                           kind="ExternalInput")
    # wrapped int16 indices, replicated across all 128 partitions:
    # [block, call, 128, CALL_IDX//16]
    IDX_ROWS = 128
    idx_d = nc.dram_tensor("idx", [N_OB, N_JC, IDX_ROWS, CALL_IDX // 16], dt.int16,
                           kind="ExternalInput")
    # parity mask expanded over the 8 components: [block, 128, N_IG*8]
    MSK_EXP = False
    msk_d = nc.dram_tensor(
        "msk", [N_OB, 128, N_IG * IN_GROUP if MSK_EXP else N_IG], dt.float32,
        kind="ExternalInput")
    # x.T packed per k-chunk: [128, 32*64], col (kc, t)
    xt_d = nc.dram_tensor("xt", [128, (IN_F // 128) * TOKENS], dt.float32,
                          kind="ExternalInput")
    scl_d = nc.dram_tensor("scl", [TOKENS, O_PAD], dt.float32, kind="ExternalInput")
    bia_d = nc.dram_tensor("bia", [TOKENS, O_PAD], dt.float32, kind="ExternalInput")
    y_d = nc.dram_tensor("y", [TOKENS, O_PAD], dt.float32, kind="ExternalOutput")

    with tile.TileContext(nc) as tc:
        with (
            tc.tile_pool(name="const", bufs=1) as cpool,
            tc.tile_pool(name="idx", bufs=int(os.environ.get("AQLM_IBUFS", "3"))) as ipool,
            tc.tile_pool(name="msk", bufs=2) as mpool,
            tc.tile_pool(name="gath", bufs=int(os.environ.get("AQLM_GBUFS", "8"))) as gpool,
            tc.tile_pool(name="w", bufs=4) as wpool,
            tc.tile_pool(name="wt", bufs=4) as wtpool,
            tc.tile_pool(name="y", bufs=2) as ypool,
            tc.tile_pool(name="pt", bufs=4, space="PSUM") as ptpool,
            tc.tile_pool(name="py", bufs=2, space="PSUM") as pypool,
        ):
            xt = cpool.tile([128, (IN_F // 128) * TOKENS], dt.float32)
            nc.sync.dma_start(xt[:], xt_d.ap())
            scl = cpool.tile([TOKENS, O_PAD], dt.float32)
            nc.sync.dma_start(scl[:], scl_d.ap())
            bia = cpool.tile([TOKENS, O_PAD], dt.float32)
            nc.sync.dma_start(bia[:], bia_d.ap())
            ident = cpool.tile([128, 128], dt.float32)
            make_identity(nc, ident[:])

            qn = 0
            for b in range(N_OB):
                # per-block index + mask staging
                idxb = ipool.tile([128, N_JC * (CALL_IDX // 16)], dt.int16, tag="idx")
                if IDX_ROWS == 128:
                    nc.sync.dma_start(
                        idxb[:].rearrange("p (c w) -> p c w", w=CALL_IDX // 16),
                        idx_d.ap()[b].rearrange("c p w -> p c w"),
                    )
                else:
                    nc.sync.dma_start(
                        idxb[16:32].rearrange("p (c w) -> p c w", w=CALL_IDX // 16),
                        idx_d.ap()[b].rearrange("c p w -> p c w"),
                    )
                mskb = mpool.tile(
                    [128, N_IG * IN_GROUP if MSK_EXP else N_IG], dt.float32, tag="msk")
                nc.sync.dma_start(mskb[:], msk_d.ap()[b])

                ypsum = pypool.tile([TOKENS, 128], dt.float32, tag="ypsum")

                for jc in range(N_JC):
                    gath = gpool.tile([128, JC, ELEM], dt.float32, tag="gath")
                    nc.gpsimd.dma_gather(
                        gath[:],
                        tab_d.ap()[qn % NTAB],
                        idxb[:, ts(jc, CALL_IDX // 16)],
                        num_idxs=CALL_IDX,
                        num_idxs_reg=CALL_IDX,
                        elem_size=ELEM,
                        single_packet=bool(int(os.environ.get("AQLM_SP", "0"))),
                        queue_num=qn % 4,
                    )
                    qn += 1

                    ge = gath[:, :, 0:IN_GROUP]
                    go = gath[:, :, IN_GROUP:2 * IN_GROUP]
                    if MSK_EXP:
                        m8 = mskb[:, ts(jc, JC * IN_GROUP)].rearrange(
                            "p (j c) -> p j c", c=IN_GROUP)
                    else:
                        m8 = mskb[:, ts(jc, JC)].rearrange(
                            "p (j one) -> p j one", one=1).broadcast_to(
                            [128, JC, IN_GROUP])

                    wd = wpool.tile([128, JC, IN_GROUP], dt.float32, tag="wd")
                    nc.vector.tensor_tensor(
                        out=wd[:], in0=go, in1=ge, op=mybir.AluOpType.subtract)
                    nc.vector.tensor_tensor(
                        out=wd[:], in0=wd[:], in1=m8, op=mybir.AluOpType.mult)
                    w8 = wpool.tile([128, JC * IN_GROUP], dt.float32, tag="w8")
                    nc.vector.tensor_tensor(
                        out=w8[:].rearrange("p (j c) -> p j c", c=IN_GROUP),
                        in0=wd[:], in1=ge, op=mybir.AluOpType.add)

                    # two 128-col sub-blocks -> transpose -> matmul accumulate
                    for h in range(JC * IN_GROUP // 128):
                        pt = ptpool.tile([128, 128], dt.float32, tag="pt")
                        nc.tensor.transpose(pt[:], w8[:, ts(h, 128)], ident[:])
                        wt = wtpool.tile([128, 128], dt.float32, tag="wt")
                        nc.vector.tensor_copy(wt[:], pt[:])
                        kc = jc * (JC * IN_GROUP // 128) + h
                        nc.tensor.matmul(
                            ypsum[:],
                            xt[:, ts(kc, TOKENS)],
                            wt[:],
                            start=(kc == 0),
                            stop=(kc == IN_F // 128 - 1),
                        )

                # y = ypsum * scales + bias
                ysb = ypool.tile([TOKENS, 128], dt.float32, tag="ysb")
                nc.vector.tensor_tensor(
                    out=ysb[:], in0=ypsum[:],
                    in1=scl[:, ts(b, 128)],
                    op=mybir.AluOpType.mult)
                nc.vector.tensor_tensor(
                    out=ysb[:], in0=ysb[:],
                    in1=bia[:, ts(b, 128)],
                    op=mybir.AluOpType.add)
                nc.sync.dma_start(y_d.ap()[:, ts(b, 128)], ysb[:])

    nc.compile()
    return nc


def _host_prep(x, codes, codebooks, scales, bias):
    """Build the paired-row table plus per-core index/mask/x tensors."""
    import os as _os
    cb = codebooks[0, :, 0, :].astype(np.float32)          # [65536, 8]
    tab1 = np.zeros((TAB_ROWS, ELEM), np.float32)
    tab1[:, 0:IN_GROUP] = cb[0::2]
    tab1[:, IN_GROUP:2 * IN_GROUP] = cb[1::2]
    ntab = int(_os.environ.get("AQLM_NTAB", "1"))
    tab = np.ascontiguousarray(np.broadcast_to(tab1, (ntab, TAB_ROWS, ELEM)))

    xt = np.ascontiguousarray(x.T).reshape(IN_F // 128, 128, TOKENS)
    xt = np.ascontiguousarray(xt.transpose(1, 0, 2)).reshape(
        128, (IN_F // 128) * TOKENS).astype(np.float32)

    c_all = codes[:, :, 0].astype(np.int64)                # [11008, 512]

    in_maps = []
    for core in range(N_CORES):
        c = np.zeros((O_PAD, N_IG), np.int64)
        c[:O_SHARD] = c_all[core * O_SHARD:(core + 1) * O_SHARD]
        idx = (c >> 1).astype(np.int16)                    # [1408, 512]
        par = (c & 1).astype(np.float32)

        # wrapped gather indices: [block, call, 16, CALL_IDX//16] -> tile to 128
        # slot i of call (b, jc): i = j*128 + p  <->  (o = 128b+p, g = 32jc+j)
        iv = idx.reshape(N_OB, 128, N_JC, JC)              # [b, p, jc, j]
        iv = iv.transpose(0, 2, 3, 1).reshape(N_OB, N_JC, CALL_IDX)  # i = j*128+p
        iv = iv.reshape(N_OB, N_JC, CALL_IDX // 16, 16).transpose(0, 1, 3, 2)
        idx_t = np.ascontiguousarray(np.tile(iv, (1, 1, 8, 1)))

        # parity mask expanded over components: [b, 128, 512*8]
        # layout must match w8 columns: col (jc, j, comp) for partition p = o-row
        mv = par.reshape(N_OB, 128, N_IG)                  # [b, p, g]
        msk = np.ascontiguousarray(mv)

        s = np.zeros((1, O_PAD), np.float32)
        s[0, :O_SHARD] = scales[core * O_SHARD:(core + 1) * O_SHARD, 0, 0, 0]
        s = np.ascontiguousarray(np.tile(s, (TOKENS, 1)))
        bi = np.zeros((1, O_PAD), np.float32)
        bi[0, :O_SHARD] = bias[core * O_SHARD:(core + 1) * O_SHARD]
        bi = np.ascontiguousarray(np.tile(bi, (TOKENS, 1)))

        in_maps.append({
            "tab": tab,
            "idx": idx_t,
            "msk": msk,
            "xt": xt,
            "scl": s,
            "bia": bi,
        })
    return in_maps


def kernel(x, codes, codebooks, scales, bias):
    from concourse import bass_utils

    x = np.asarray(x)
    codes = np.asarray(codes)
    codebooks = np.asarray(codebooks)
    scales = np.asarray(scales)
    bias = np.asarray(bias)

    if "nc" not in _CACHED:
        _CACHED["nc"] = _build_program()
    nc = _CACHED["nc"]

    in_maps = _host_prep(x, codes, codebooks, scales, bias)
    res = bass_utils.run_bass_kernel_spmd(
        nc, in_maps, core_ids=list(range(N_CORES)))
    _CACHED["last_results"] = res

    out = np.empty((TOKENS, OUT_F), np.float32)
    for core in range(N_CORES):
        out[:, core * O_SHARD:(core + 1) * O_SHARD] = \
            res.results[core]["y"][:, :O_SHARD]
    return out

